# revision 14
# baseline (speedup 1.0000x reference)
"""Cross-attention kernel for Trainium2, 8 NeuronCores, data-parallel over batch.

Reference computes (B=64, S=512, D=1024):
    q1 = x1 @ Wq1.T + bq1
    k2 = x2 @ Wk2.T + bk2
    v2 = x2 @ Wv2.T + bv2
    attn = softmax(q1 @ k2.T, axis=-1)          # [B, S1, S2]
    out  = sum_q (attn @ v2)                    # [B, D]
(k1, v1, q2 are computed by the reference module but unused.)

Algebraic restructuring used here:
  * scores = x1 M x2.T + u[q] 1.T + 1 v[k].T + c,   M = Wq1.T Wk2
    Row-constant terms (u, c) cancel inside softmax, so
      attn = softmax_rows(x1 M x2.T + 1 v.T),  v = x2 @ vv,  vv = Wk2.T bq1.
    The 1 v.T term is folded into the M-stage for free: with
    P1' = x1 M + 1 vv.T (per-partition ACT bias during the PSUM->SBUF
    copy of P1^T), G = P1' x2.T already includes it.
  * out[b] = colsum[b] @ v2[b] with colsum[b,k] = sum_q attn[b,q,k]
           = (colsum[b] @ x2[b]) @ Wv2.T + S1 * bv2
    because each softmax row sums to 1. The S1*bv2 bias is added on host.
  * colsum is computed on the PE as E.T @ (1/Z) where E = exp(scores - rowmax),
    Z = row sums of E — no normalized attention matrix is ever materialized.
  * t[b] = colsum @ x2 is accumulated as tiny [128,1]-column matmuls into a
    persistent PSUM tile (t^T layout), so the finale out = tallT.T @ Wv2.T
    is a single 16-matmul pass at the end with no transposes.

Device work per batch: A = (x1 M)^T (64 matmuls, 512 rows each),
G = P1 x2^T (32 matmuls), softmax stats on ACT/DVE, and tiny colsum/t
matvecs. Everything else is O(D^2) host prep.
"""

import sys

import numpy as np

sys.path.insert(0, "/opt/trn_rl_repo")

B, S, D = 64, 512, 1024
NCORES = 8
BPC = B // NCORES  # batches per core
P = 128
DT = D // P  # 8 feature tiles
ST = S // P  # 4 sequence tiles
NB = 512     # PSUM bank free-dim limit for f32

_CACHED = {}


def _build_program():
    import concourse.bass as bass
    import concourse.mybir as mybir
    import concourse.tile as tile
    from contextlib import ExitStack

    f32 = mybir.dt.float32
    f32r = mybir.dt.float32r
    bf16 = mybir.dt.bfloat16
    AX = mybir.AxisListType
    AF = mybir.ActivationFunctionType

    nc = bass.Bass(trn_type="TRN2")

    # float32r (FP22-truncated reads in the PE) for the two big matmul
    # chains. The BIR verifier requires f32r-consumed tensors to be
    # *produced* as f32r, so the dtype is set on the DRAM tensors / SBUF
    # tiles themselves (same 4-byte layout).
    fbig = f32r

    x1t_d = nc.dram_tensor("x1t", [BPC, D, S], bf16, kind="ExternalInput")
    x2t_d = nc.dram_tensor("x2t", [BPC, D, S], fbig, kind="ExternalInput")
    x2n_d = nc.dram_tensor("x2n", [BPC, S, D], f32, kind="ExternalInput")
    mmat_d = nc.dram_tensor("mmat", [D, D], bf16, kind="ExternalInput")
    vvx_d = nc.dram_tensor("vvx", [P, DT], f32, kind="ExternalInput")
    wv2t_d = nc.dram_tensor("wv2t", [D, D], bf16, kind="ExternalInput")
    out_d = nc.dram_tensor("outT", [D, BPC], f32, kind="ExternalOutput")

    with ExitStack() as ctx:
        tc = ctx.enter_context(tile.TileContext(nc))
        singles = ctx.enter_context(tc.tile_pool(name="singles", bufs=1))
        xpool = ctx.enter_context(tc.tile_pool(name="xpool", bufs=2))
        work = ctx.enter_context(tc.tile_pool(name="work", bufs=2))
        ps_a = ctx.enter_context(tc.tile_pool(name="ps_a", bufs=2, space="PSUM"))
        ps_g = ctx.enter_context(tc.tile_pool(name="ps_g", bufs=2, space="PSUM"))
        ps_t = ctx.enter_context(tc.tile_pool(name="ps_t", bufs=1, space="PSUM"))
        ps_s = ctx.enter_context(tc.tile_pool(name="ps_s", bufs=1, space="PSUM"))

        # ---- constants resident in SBUF ----
        m_sb = singles.tile([P, DT, D], bf16)   # M[d,e]: m_sb[p,t,e] = M[t*P+p, e]
        vv_sb = singles.tile([P, DT], f32)      # vv[e] in e-partition layout
        nc.sync.dma_start(out=vv_sb, in_=vvx_d[:])
        wv2_sb = singles.tile([P, DT, D], bf16)  # Wv2T[e,e'] rows, bf16
        tall_sb = singles.tile([P, BPC, DT], bf16)  # t^T[e, b] columns
        tall_ps = ps_t.tile([P, BPC, DT], f32)      # persistent PSUM accumulator
        outT_ps = ps_s.tile([P, DT, BPC], f32)      # persistent out^T accumulator
        warm_sb = singles.tile([P, NB], bf16)       # zeros; PE pstate warmup
        nc.vector.memset(warm_sb, 0.0)
        warm_ps = ps_g.tile([P, NB], f32, tag="g", name="warm")
        for _ in range(6):
            # dummy matmuls keep the PE busy (and its p-state ramping)
            # while the cold-start DMAs land
            nc.tensor.matmul(warm_ps, lhsT=warm_sb[:, 0:P], rhs=warm_sb,
                             start=True, stop=True)

        st = {}

        def phase_a(b, mid=None):
            x1t_sb = xpool.tile([P, DT, S], bf16, tag="x1t", name=f"x1t_{b}")
            if b == 0:
                # Interleave mmat and x1t half-loads so the first matmuls
                # can start as soon as the first chunk lands, instead of
                # stalling for the full cold DMA. DMA issue costs ~650ns
                # per instruction on the queue, so chunk coarsely.
                H = DT // 4
                for c in range(4):
                    ks = slice(c * H, (c + 1) * H)
                    rs = slice(c * H * P, (c + 1) * H * P)
                    nc.sync.dma_start(
                        out=m_sb[:, ks, :],
                        in_=mmat_d[rs, :].rearrange("(t p) e -> p t e", p=P),
                    )
                    nc.sync.dma_start(
                        out=x1t_sb[:, ks, :],
                        in_=x1t_d[b, rs, :].rearrange("(t p) s -> p t s", p=P),
                    )
            else:
                nc.sync.dma_start(
                    out=x1t_sb, in_=x1t_d[b].rearrange("(t p) s -> p t s", p=P)
                )
            x2t_sb = xpool.tile([P, DT, S], fbig, tag="x2t", name=f"x2t_{b}")
            x2n_sb = xpool.tile([P, ST, D], f32, tag="x2n", name=f"x2n_{b}")

            # b==0: same SP queue as the cold mmat/x1t chunks — per-queue
            # FIFO keeps them off the DMA wire until the chunks the PE is
            # waiting on have landed. b>0: ACT queue, issue in parallel.
            eng = nc.sync if b <= 1 else nc.scalar
            eng.dma_start(
                out=x2t_sb, in_=x2t_d[b].rearrange("(t p) s -> p t s", p=P)
            )
            if b == 1:
                # wv2 is needed by fin(0) (~after G(2)); slot it here so it
                # lands in time but never contends with the latency-critical
                # cold loads or x1t(1).
                nc.sync.dma_start(
                    out=wv2_sb,
                    in_=wv2t_d[:].rearrange("(t p) e -> p t e", p=P),
                )
            eng.dma_start(
                out=x2n_sb, in_=x2n_d[b].rearrange("(t p) e -> p t e", p=P)
            )
            st[("x2t", b)] = x2t_sb
            st[("x2n", b)] = x2n_sb

            # P1T[e,s] = sum_d M[d,e] * x1T[d,s]   ((x1 @ M)^T)
            p1t_sb = work.tile([P, DT, S], fbig, tag="p1t", name=f"p1t_{b}")
            for m in range(DT):
                p1_ps = ps_a.tile([P, NB], f32, tag="big", name=f"p1ps_{b}_{m}")
                for k in range(DT):
                    nc.tensor.matmul(
                        p1_ps,
                        lhsT=m_sb[:, k, m * P : (m + 1) * P],
                        rhs=x1t_sb[:, k, :],
                        start=(k == 0),
                        stop=(k == DT - 1),
                    )
                # PSUM->SBUF copy on ACT with the per-partition vv bias:
                # p1t[e,q] = P1T[e,q] + vv[e]
                nc.scalar.activation(
                    out=p1t_sb[:, m, :],
                    in_=p1_ps,
                    func=AF.Identity,
                    bias=vv_sb[:, m : m + 1],
                    scale=1.0,
                )
                if m == DT - 3 and mid is not None:
                    # emit the previous batch's tiny colsum/t matvecs here so
                    # their DVE-copy round-trip latency hides behind the
                    # remaining A matmuls instead of stalling the PE between
                    # A(b) and G(b)
                    mid()
            st[("p1t", b)] = p1t_sb

        def phase_g(b):
            # G[q,j] = sum_e P1T[e,q] x2T[e,j]; row softmax stats
            p1t_sb = st.pop(("p1t", b))
            x2t_sb = st.pop(("x2t", b))
            e_sb = work.tile([P, ST, S], f32, tag="esb", name=f"e_{b}")
            wr_sb = work.tile([P, ST], f32, tag="wrecip", name=f"wr_{b}")
            for m in range(ST):
                g_ps = ps_g.tile([P, NB], f32, tag="g", name=f"gps_{b}_{m}")
                for k in range(DT):
                    nc.tensor.matmul(
                        g_ps,
                        lhsT=p1t_sb[:, k, m * P : (m + 1) * P],
                        rhs=x2t_sb[:, k, :],
                        start=(k == 0),
                        stop=(k == DT - 1),
                    )
                nmax_sb = work.tile([P, 1], f32, tag="nmax", name=f"nm_{b}_{m}")
                nc.vector.reduce_max(out=nmax_sb, in_=g_ps, axis=AX.X, negate=True)
                z_sb = work.tile([P, 1], f32, tag="z", name=f"z_{b}_{m}", bufs=4)
                nc.scalar.activation(
                    out=e_sb[:, m, :],
                    in_=g_ps,
                    func=AF.Exp,
                    bias=nmax_sb,
                    scale=1.0,
                    accum_out=z_sb,
                )
                nc.vector.reciprocal(wr_sb[:, m : m + 1], z_sb)
            st[("e", b)] = e_sb
            st[("wr", b)] = wr_sb

        def phase_cs(b):
            # colsumT[k2] = sum_q E[q,k2] * (1/Z[q])
            e_sb = st.pop(("e", b))
            wr_sb = st.pop(("wr", b))
            cs_sb = work.tile([P, ST], f32, tag="cs", name=f"cs_{b}")
            cs_ps = ps_s.tile([P, ST], f32, tag="small", name=f"csps_{b}")
            for m in range(ST):
                for k in range(ST):
                    nc.tensor.matmul(
                        cs_ps[:, m : m + 1],
                        lhsT=e_sb[:, k, m * P : (m + 1) * P],
                        rhs=wr_sb[:, k : k + 1],
                        start=(k == 0),
                        stop=(k == ST - 1),
                    )
            nc.vector.tensor_copy(cs_sb, cs_ps)
            st[("cs", b)] = cs_sb

        def phase_t(b):
            # t^T[e] = sum_j x2n[j,e] colsum[j], accumulated as [128,1]
            # columns of the persistent tall tile (e-partition layout).
            cs_sb = st.pop(("cs", b))
            x2n_sb = st.pop(("x2n", b))
            for m in range(DT):
                for k in range(ST):
                    nc.tensor.matmul(
                        tall_ps[:, b, m : m + 1],
                        lhsT=x2n_sb[:, k, m * P : (m + 1) * P],
                        rhs=cs_sb[:, k : k + 1],
                        start=(k == 0),
                        stop=(k == ST - 1),
                    )
            nc.vector.tensor_copy(tall_sb[:, b, :], tall_ps[:, b, :])

        def phase_fin(b):
            # out^T[e',b] column: sum_e Wv2T[e,e'] * tallT[e,b] as tiny
            # [128,1] accumulations — engine cost ~2ns each, fully hidden
            # behind the big matmuls; nothing left for the kernel tail.
            for mp in range(DT):
                for k in range(DT):
                    nc.tensor.matmul(
                        outT_ps[:, mp, b : b + 1],
                        lhsT=wv2_sb[:, k, mp * P : (mp + 1) * P],
                        rhs=tall_sb[:, b, k : k + 1],
                        start=(k == 0),
                        stop=(k == DT - 1),
                    )

        for b in range(BPC):
            if b > 0:
                phase_a(b, mid=lambda bb=b: (phase_cs(bb - 1), phase_t(bb - 1)))
            else:
                phase_a(b)
            phase_g(b)
            if b > 1:
                phase_fin(b - 2)
        phase_cs(BPC - 1)
        phase_t(BPC - 1)
        phase_fin(BPC - 2)
        phase_fin(BPC - 1)

        outT_sb = singles.tile([P, DT, BPC], f32)
        nc.vector.tensor_copy(outT_sb, outT_ps)
        nc.sync.dma_start(
            out=out_d[:].rearrange("(t p) b -> p t b", p=P), in_=outT_sb
        )

    return nc


def _split_multi_waits(nc):
    """Walrus in this toolchain rejects >1 sync-wait per instruction
    ("Too many sync wait commands"). Move extra waits onto dedicated
    EventSemaphore carrier instructions inserted just before the owner on
    the same engine — the sequencer satisfies them in program order, so
    semantics are identical."""
    import concourse.mybir as mybir

    n = 0
    for fn in nc.m.functions:
        for blk in fn.blocks:
            out = []
            for inst in blk.instructions:
                si = inst.sync_info
                if si is not None:
                    waits = list(si.on_wait or [])
                    if len(waits) > 1:
                        for w in waits[:-1]:
                            n += 1
                            out.append(
                                mybir.InstEventSemaphore(
                                    name=f"wsplit-{n}",
                                    engine=inst.engine,
                                    sync_info=mybir.SyncInfo(
                                        on_wait=[w], on_update=[]
                                    ),
                                )
                            )
                        si.on_wait = waits[-1:]
                out.append(inst)
            blk.instructions = out
    return n


def _get_program():
    if "nc" not in _CACHED:
        nc = _build_program()
        _split_multi_waits(nc)
        _CACHED["nc"] = nc
    return _CACHED["nc"]


def kernel(input1, input2,
           W_q1, b_q1, W_k1, b_k1, W_v1, b_v1,
           W_q2, b_q2, W_k2, b_k2, W_v2, b_v2,
           _want_trace=False):
    import ml_dtypes
    from concourse.bass_utils import run_bass_kernel_spmd

    f64 = np.float64
    mmat = (W_q1.astype(f64).T @ W_k2.astype(f64)).astype(ml_dtypes.bfloat16)
    vv = (W_k2.astype(f64).T @ b_q1.astype(f64)).astype(np.float32)
    vvx = np.ascontiguousarray(vv.reshape(DT, P).T)  # vvx[p,t] = vv[t*P+p]
    wv2t = np.ascontiguousarray(W_v2.T.astype(ml_dtypes.bfloat16))

    input1 = np.ascontiguousarray(input1, dtype=np.float32)
    input2 = np.ascontiguousarray(input2, dtype=np.float32)
    x1t = np.ascontiguousarray(input1.transpose(0, 2, 1).astype(ml_dtypes.bfloat16))
    x2t = np.ascontiguousarray(input2.transpose(0, 2, 1))

    nc = _get_program()

    in_maps = []
    for c in range(NCORES):
        lo, hi = c * BPC, (c + 1) * BPC
        in_maps.append(
            {
                "x1t": x1t[lo:hi],
                "x2t": x2t[lo:hi],
                "x2n": input2[lo:hi],
                "mmat": mmat,
                "vvx": vvx,
                "wv2t": wv2t,
            }
        )

    res = run_bass_kernel_spmd(
        nc, in_maps, core_ids=list(range(NCORES)), trace=_want_trace
    )
    bias = (float(S) * b_v2.astype(f64)).astype(np.float32)
    out = np.concatenate([r["outT"].T for r in res.results], axis=0) + bias
    if _want_trace:
        return out, res
    return out
